# revision 2
# baseline (speedup 1.0000x reference)
"""v7: U^T restructure + fp8 DoubleRow 3-term projections.

Per core (1 batch x 8 heads): K/Q/V projections via fp8 hi/lo DoubleRow
(3-term, host-packed x*8 / W*32), S = K^T q-major bf16 as v6, exp on ACT,
U^T[q, d] = sum_j E_j^T V_j with q on partitions (65-wide rhs: col 64 =
256.0 denominator ride-along), per-head PSUM accumulator [128, 4qt x 65],
DVE reciprocal + tensor_scalar normalization, PE transposes -> O^T, Y^T
from bf16 O^T @ bf16 Wo.

PSUM: st [128,2048] quarters (4 banks, 2j ping-pong) + u 2x[128,260] +
jobs 2x[128,512] = 8 banks.
"""

import numpy as np

B = 4
L = 2048
D = 1024
INNER = 1024
HEADS = 16
DH = 64
N_CORES = 8
IH = INNER // 2
SCALE = DH ** -0.5
XSC = 8.0    # host x scale
WSC = 32.0   # host W scale
PS = XSC * WSC  # projection output scale (256)

_CACHE = {}


def _build_nc():
    import os
    VAR = int(os.environ.get("V7_VARIANT", "0"))
    import concourse.bass as bass
    import concourse.tile as tile
    from concourse import bacc, mybir
    from concourse.masks import make_identity

    f32 = mybir.dt.float32
    bf16 = mybir.dt.bfloat16
    fp8 = mybir.dt.float8e4
    DR = mybir.MatmulPerfMode.DoubleRow
    P = 128
    L_, D_, IH_, DH_ = L, D, IH, DH
    LQB = 512
    XS = 512
    NH = IH_ // DH_        # 8 heads per core
    NPAIR = NH // 2        # 4
    NJT = L_ // P          # 16 j tiles
    NQT = LQB // P         # 4 q subtiles per block
    NLQB = L_ // LQB       # 4
    NDT = D_ // P          # 8
    NIT = IH_ // P         # 4 inner tiles (pairs)
    NXS = L_ // XS         # 4
    VW = DH_ + 1           # 65: v + denominator column

    nc = bacc.Bacc("TRN2", target_bir_lowering=False, debug=False)
    # x: [p, s*2*NDT*XS + hl*NDT*XS + d*XS + c] fp8 (hi|lo per slice)
    x1 = nc.declare_dram_parameter("x1h", [P, NXS * 2 * NDT * XS], fp8,
                                   isOutput=False)
    x2 = nc.declare_dram_parameter("x2h", [P, NXS * 2 * NDT * XS], fp8,
                                   isOutput=False)
    # wq/wk m-major: [p, m*2*NDT*P + hl*NDT*P + d*P + c] fp8
    wq = nc.declare_dram_parameter("wqh", [P, NIT * 2 * NDT * P], fp8,
                                   isOutput=False)
    wk = nc.declare_dram_parameter("wkh", [P, NIT * 2 * NDT * P], fp8,
                                   isOutput=False)
    # wv: [p, hl*NDT*IH + d*IH + c] fp8
    wv = nc.declare_dram_parameter("wvh", [P, 2 * NDT * IH_], fp8,
                                   isOutput=False)
    wo = nc.declare_dram_parameter("woh", [P, NIT * D_], bf16, isOutput=False)
    yt = nc.declare_dram_parameter("yt", [D_, L_], f32, isOutput=True)

    with tile.TileContext(nc) as tc:
        with tc.tile_pool(name="persist", bufs=1) as persist:
            kt = persist.tile([P, NIT * L_], bf16, name="kt")
            qt = persist.tile([P, NIT * L_], bf16, name="qt")
            vva = persist.tile([P, NJT * NH * VW], bf16, name="vva")
            ident = persist.tile([P, P], bf16, name="ident")
            wkt = persist.tile([P, NIT * 2 * NDT * P], fp8, name="wkt")
            wqt = persist.tile([P, NIT * 2 * NDT * P], fp8, name="wqt")
            wvt = persist.tile([P, 2 * NDT * IH_], fp8, name="wvt")
            wot = persist.tile([P, NIT * D_], bf16, name="wot")
            make_identity(nc, ident)
            # denominator ride-along columns (col 64 of each [.,j,h,65])
            vvav = vva.rearrange("p (j h c) -> p j h c", h=NH, c=VW)
            nc.vector.memset(vvav[:, :, :, DH_:], float(PS))

            with (
                tc.tile_pool(name="xts2", bufs=1) as xts2,
                tc.tile_pool(name="xts1", bufs=2) as xts1,
                tc.tile_pool(name="stj", bufs=2, space="PSUM") as stjp,
                tc.tile_pool(name="upool", bufs=2, space="PSUM") as upool,
                tc.tile_pool(name="jobs", bufs=2, space="PSUM") as jobs,
                tc.tile_pool(name="ets", bufs=8) as ets,
                tc.tile_pool(name="smalls", bufs=2) as smalls,
                tc.tile_pool(name="osbp", bufs=4) as osbp,
                tc.tile_pool(name="otsb", bufs=8) as otsbp,
                tc.tile_pool(name="youts", bufs=6) as youts,
            ):
                st = None

                # ---- DMA: deadline-ordered, hi-halves first ----
                xt2, xt1s = {}, {}
                XB = 2 * NDT * XS          # per-slice block (8192)
                HB = NDT * XS              # hi/lo half (4096)
                WB = 2 * NDT * P           # per-m block (2048)
                WH = NDT * P               # 1024

                def load_x2_slice(s):
                    t = xts2.tile([P, XB], fp8, name="x2t", tag=f"x2_{s}",
                                  bufs=1)
                    xt2[s] = t
                    nc.sync.dma_start(out=t, in_=x2[:, s * XB:(s + 1) * XB])

                def load_x1_slice(sl):
                    t = xts1.tile([P, XB], fp8, name="x1t", tag="x1", bufs=2)
                    xt1s[sl] = t
                    nc.sync.dma_start(out=t, in_=x1[:, sl * XB:(sl + 1) * XB])

                nc.sync.dma_start(out=wkt[:, 0:WB], in_=wk[:, 0:WB])
                nc.sync.dma_start(out=wqt[:, 0:WB], in_=wq[:, 0:WB])
                t0 = xts2.tile([P, XB], fp8, name="x2t", tag="x2_0", bufs=1)
                xt2[0] = t0
                nc.sync.dma_start(out=t0[:, 0:HB], in_=x2[:, 0:HB])
                nc.sync.dma_start(out=t0[:, HB:], in_=x2[:, HB:XB])
                nc.sync.dma_start(out=wvt[:, 0:HB], in_=wv[:, 0:HB])
                t1 = xts1.tile([P, XB], fp8, name="x1t", tag="x1", bufs=2)
                xt1s[0] = t1
                nc.sync.dma_start(out=t1[:, 0:HB], in_=x1[:, 0:HB])
                nc.sync.dma_start(out=t1[:, HB:], in_=x1[:, HB:XB])
                nc.sync.dma_start(out=wvt[:, HB:], in_=wv[:, HB:])
                load_x2_slice(1)
                load_x2_slice(2)
                load_x2_slice(3)
                nc.sync.dma_start(out=wqt[:, WB:2 * WB], in_=wq[:, WB:2 * WB])
                nc.sync.dma_start(out=wkt[:, WB:2 * WB], in_=wk[:, WB:2 * WB])
                nc.sync.dma_start(out=wqt[:, 2 * WB:], in_=wq[:, 2 * WB:])
                nc.sync.dma_start(out=wkt[:, 2 * WB:], in_=wk[:, 2 * WB:])
                nc.sync.dma_start(out=wot, in_=wo[:, :])

                # ---- fp8 DR 3-term projection jobs ----
                def kjob(m, s):
                    t = jobs.tile([P, XS], f32, name="kj", tag="job")
                    wv_ = wkt.rearrange("p (m z d c) -> p m z d c", m=NIT,
                                        z=2, c=P)
                    xv_ = xt2[s].rearrange("p (z d c) -> p z d c", z=2, c=XS)
                    n = 0
                    for whl, xhl in ((0, 0), (0, 1), (1, 0)):
                        for d in range(0, NDT, 2):
                            nc.tensor.matmul(
                                t, lhsT=wv_[:, m, whl, d:d + 2, :],
                                rhs=xv_[:, xhl, d:d + 2, :],
                                start=(n == 0), stop=(n == 11), perf_mode=DR)
                            n += 1
                    nc.vector.tensor_copy(
                        kt[:, m * L_ + s * XS: m * L_ + (s + 1) * XS], t)

                def qjob(sl, m):
                    t = jobs.tile([P, XS], f32, name="qj", tag="job")
                    wv_ = wqt.rearrange("p (m z d c) -> p m z d c", m=NIT,
                                        z=2, c=P)
                    xv_ = xt1s[sl].rearrange("p (z d c) -> p z d c", z=2, c=XS)
                    n = 0
                    for whl, xhl in ((0, 0), (0, 1), (1, 0)):
                        for d in range(0, NDT, 2):
                            nc.tensor.matmul(
                                t, lhsT=wv_[:, m, whl, d:d + 2, :],
                                rhs=xv_[:, xhl, d:d + 2, :],
                                start=(n == 0), stop=(n == 11), perf_mode=DR)
                            n += 1
                    nc.vector.tensor_copy(
                        qt[:, m * L_ + sl * XS: m * L_ + (sl + 1) * XS], t)

                def vjob(s, tq):
                    t = jobs.tile([P, IH_], f32, name="vj", tag="job")
                    xv_ = xt2[s].rearrange("p (z d c) -> p z d c", z=2, c=XS)
                    wv_ = wvt.rearrange("p (z d c) -> p z d c", z=2, c=IH_)
                    n = 0
                    for xhl, whl in ((0, 0), (1, 0), (0, 1)):
                        for d in range(0, NDT, 2):
                            nc.tensor.matmul(
                                t,
                                lhsT=xv_[:, xhl, d:d + 2, tq * P:(tq + 1) * P],
                                rhs=wv_[:, whl, d:d + 2, :],
                                start=(n == 0), stop=(n == 11), perf_mode=DR)
                            n += 1
                    j = s * (XS // P) + tq
                    dst = (vva[:, j * NH * VW:(j + 1) * NH * VW]
                           .rearrange("p (h c) -> p h c", c=VW)[:, :, :DH_])
                    nc.vector.tensor_copy(
                        dst, t.rearrange("p (h c) -> p h c", c=DH_))

                def run_job(spec):
                    kind = spec[0]
                    if kind == "k":
                        kjob(spec[1], spec[2])
                    elif kind == "v":
                        vjob(spec[1], spec[2])
                    elif kind == "q":
                        qjob(spec[1], spec[2])

                # ---- attention ----
                ROT = [(0, 1), (2, 3)]  # st quarter pairs, ping-pong by jg

                def emit_s(hp, j, lqb, jg):
                    stt = stjp.tile([P, 2 * LQB], f32, name="stt", tag="st")
                    base = 0
                    for hh in range(2):
                        nc.tensor.matmul(
                            stt[:, base + hh * LQB: base + (hh + 1) * LQB],
                            lhsT=kt[hh * DH_:(hh + 1) * DH_,
                                    hp * L_ + j * P: hp * L_ + (j + 1) * P],
                            rhs=qt[hh * DH_:(hh + 1) * DH_,
                                   hp * L_ + lqb * LQB:
                                   hp * L_ + lqb * LQB + LQB],
                            start=True, stop=True)
                    et = ets.tile([P, 2 * LQB], bf16, name="et")
                    nc.scalar.activation(
                        et, stt[:, base: base + 2 * LQB],
                        mybir.ActivationFunctionType.Exp,
                        scale=float(SCALE / (PS * PS)))
                    return et

                def emit_u(hp, et, j, u0, u1):
                    if VAR >= 4:
                        return
                    for hh, u in ((0, u0), (1, u1)):
                        h = 2 * hp + hh
                        for qn in range(NQT):
                            nc.tensor.matmul(
                                u[:, qn * VW:(qn + 1) * VW],
                                lhsT=et[:, hh * LQB + qn * P:
                                        hh * LQB + (qn + 1) * P],
                                rhs=vva[:, (j * NH + h) * VW:
                                        (j * NH + h + 1) * VW],
                                start=(j == 0 and qn == 0),
                                stop=(j == NJT - 1 and qn == NQT - 1))

                def drain_pair(hp, u0, u1, lqb, otsb_cur):
                    if VAR >= 2:
                        return
                    # stage u -> SBUF fast (frees the psum banks), then
                    # normalize from the staged copy
                    us = smalls.tile([P, 2 * NQT * VW], f32, name="us",
                                     tag="us", bufs=2)
                    nc.vector.tensor_copy(us[:, 0:NQT * VW], u0)
                    nc.vector.tensor_copy(us[:, NQT * VW:], u1)
                    rcp = smalls.tile([P, 2 * NQT], f32, name="rcp",
                                      tag="rcp", bufs=2)
                    usv = us.rearrange("p (q c) -> p q c", c=VW)
                    nc.vector.reciprocal(
                        rcp, usv[:, :, DH_])
                    osbs = []
                    for qn in range(NQT):
                        osb = osbp.tile([P, 2 * DH_], bf16, name="osb")
                        osbs.append(osb)
                        nc.vector.tensor_scalar_mul(
                            osb[:, 0:DH_], usv[:, qn, 0:DH_],
                            rcp[:, qn:qn + 1])
                        nc.vector.tensor_scalar_mul(
                            osb[:, DH_:], usv[:, NQT + qn, 0:DH_],
                            rcp[:, NQT + qn:NQT + qn + 1])
                    if VAR >= 1:
                        return
                    tdefer.append((osbs, otsb_cur[hp]))

                def flush_transposes():
                    if not tdefer:
                        return
                    osbs, ot = tdefer.pop(0)
                    for qn in range(NQT):
                        for hh in range(2):
                            tp = jobs.tile([DH_, P], bf16, name="tp",
                                           tag="job")
                            nc.tensor.matmul(
                                tp, lhsT=osbs[qn][:, hh * DH_:(hh + 1) * DH_],
                                rhs=ident, is_transpose=True)
                            nc.vector.tensor_copy(
                                ot[hh * DH_:(hh + 1) * DH_,
                                   qn * P:(qn + 1) * P], tp)

                def yjob(otsb_prev, lqb_prev, dsub, py=None, it0=0):
                    if VAR >= 3:
                        return
                    if py is None:
                        py = jobs.tile([P, LQB], f32, name="py", tag="job")
                    for it in range(it0, NIT):
                        nc.tensor.matmul(
                            py[:, 0:LQB],
                            lhsT=wot[:, it * D_ + dsub * P:
                                     it * D_ + (dsub + 1) * P],
                            rhs=otsb_prev[it],
                            start=(it == 0), stop=(it == NIT - 1))
                    yo = youts.tile([P, LQB], f32, name="yo")
                    nc.vector.tensor_copy(yo, py[:, 0:LQB])
                    nc.sync.dma_start(
                        out=yt[dsub * P:(dsub + 1) * P,
                               lqb_prev * LQB:(lqb_prev + 1) * LQB],
                        in_=yo)

                def yjob_partial(otsb_prev, dsub, pool_tile):
                    if VAR >= 3:
                        return
                    for it in range(NIT - 1):
                        nc.tensor.matmul(
                            pool_tile[:, 0:LQB],
                            lhsT=wot[:, it * D_ + dsub * P:
                                     it * D_ + (dsub + 1) * P],
                            rhs=otsb_prev[it],
                            start=(it == 0), stop=False)

                # block-0 job inserts at (pair, j) slots
                b0_ins = {
                    (0, 0): [("v", 0, 0)], (0, 1): [("v", 0, 1)],
                    (0, 2): [("v", 0, 2)], (0, 3): [("v", 0, 3), ("k", 0, 1)],
                    (0, 4): [("v", 1, 0)], (0, 5): [("v", 1, 1)],
                    (0, 6): [("v", 1, 2), ("k", 0, 2)], (0, 7): [("v", 1, 3)],
                    (0, 8): [("v", 2, 0)], (0, 9): [("v", 2, 1)],
                    (0, 10): [("v", 2, 2), ("k", 0, 3)], (0, 11): [("v", 2, 3)],
                    (0, 12): [("v", 3, 0)], (0, 13): [("v", 3, 1)],
                    (0, 14): [("v", 3, 2), ("q", 0, 1)],
                    (0, 15): [("v", 3, 3), ("k", 1, 0)],
                    (1, 0): [("k", 1, 1)], (1, 2): [("k", 1, 2)],
                    (1, 4): [("k", 1, 3)], (1, 6): [("k", 2, 0)],
                    (1, 8): [("k", 2, 1)], (1, 10): [("k", 2, 2)],
                    (1, 12): [("k", 2, 3)], (1, 14): [("q", 0, 2)],
                    (2, 0): [("k", 3, 0)], (2, 2): [("k", 3, 1)],
                    (2, 4): [("k", 3, 2)], (2, 6): [("k", 3, 3)],
                    (2, 8): [("q", 0, 3)], (2, 12): [("q", 1, 0)],
                    (3, 2): [("q", 1, 1)], (3, 6): [("q", 1, 2)],
                    (3, 10): [("q", 1, 3)],
                }


                # prologue
                kjob(0, 0)
                qjob(0, 0)

                jg = 0
                tdefer = []    # deferred transpose batches
                pend = []      # (hp, et, j, u0, u1, lqb, otsb_cur)
                drains = []    # pairs awaiting drain after last U pop
                ydefer = []
                otsb_blocks = []  # per lqb: dict pair -> ot tile
                partials = []

                for lqb in range(NLQB):
                    if lqb < NLQB - 1:
                        load_x1_slice(lqb + 1)
                    otsb_cur = {hp: otsbp.tile([P, LQB], bf16, name="ot")
                                for hp in range(NPAIR)}
                    otsb_blocks.append(otsb_cur)
                    for hp in range(NPAIR):
                        u0 = upool.tile([P, NQT * VW], f32, name="u0",
                                        tag="u")
                        u1 = upool.tile([P, NQT * VW], f32, name="u1",
                                        tag="u")
                        for j in range(NJT):
                            et = emit_s(hp, j, lqb, jg)
                            jg += 1
                            pend.append((hp, et, j, u0, u1, lqb, otsb_cur))
                            while len(pend) > 4:
                                e = pend.pop(0)
                                emit_u(e[0], e[1], e[2], e[3], e[4])
                                if e[2] == NJT - 1:
                                    drain_pair(e[0], e[3], e[4], e[5], e[6])
                            if j == 9:
                                flush_transposes()
                            if (lqb == NLQB - 1 and hp == NPAIR - 1
                                    and j == 14 and VAR == 0):
                                ds = 4
                                pj = jobs.tile([P, LQB], f32, name="py",
                                               tag="job")
                                yjob_partial(
                                    [otsb_blocks[-1][i] for i in range(3)]
                                    + [None], ds, pj)
                                partials.append((ds, pj))
                            if VAR >= 5:
                                pass
                            elif lqb == 0:
                                for spec in b0_ins.get((hp, j), ()):
                                    run_job(spec)
                            else:
                                if j in (10, 13) and ydefer:
                                    yjob(*ydefer.pop(0))
                                if j == 8 and lqb < NLQB - 1:
                                    qjob(lqb + 1, hp)
                        if lqb == NLQB - 1 and hp == NPAIR - 1:
                            # flush: last pair's U + drain
                            while pend:
                                e = pend.pop(0)
                                emit_u(e[0], e[1], e[2], e[3], e[4])
                                if e[2] == NJT - 1:
                                    if e[0] == NPAIR - 1:
                                        # partials on freed st slots
                                        prev = otsb_blocks[-1]
                                        pprev = [prev[i] for i in range(3)]
                                        for sl in range(2):
                                            big = stjp.tile(
                                                [P, 2 * LQB], f32,
                                                name="stt", tag="st")
                                            for hh2 in range(2):
                                                ds = sl * 2 + hh2
                                                pt = big[:, hh2 * LQB:
                                                         (hh2 + 1) * LQB]
                                                yjob_partial(pprev + [None],
                                                             ds, pt)
                                                partials.append((ds, pt))
                                    drain_pair(e[0], e[3], e[4], e[5], e[6])
                    skip = ()
                    if lqb == NLQB - 1:
                        skip = tuple(range(D_ // P))
                    ydefer.extend(
                        ([otsb_blocks[lqb][i] for i in range(NPAIR)], lqb, ds)
                        for ds in range(D_ // P) if ds not in skip)

                while tdefer:
                    flush_transposes()
                # tail: finish partials (it=3) then remaining yjobs
                last = [otsb_blocks[-1][i] for i in range(NPAIR)]
                for ds, pt in partials:
                    yjob(last, NLQB - 1, ds, py=pt, it0=NIT - 1)
                for ds in range(5, D_ // P):
                    yjob(last, NLQB - 1, ds)
                while ydefer:
                    yjob(*ydefer.pop(0))
    nc.compile()
    return nc


def _get_nc():
    if "nc" not in _CACHE:
        _CACHE["nc"] = _build_nc()
    return _CACHE["nc"]


def _f8():
    import ml_dtypes
    return ml_dtypes.float8_e4m3


def _hi_lo(a):
    f8 = _f8()
    h = a.astype(f8)
    l = (a - h.astype(np.float32)).astype(f8)
    return h, l


def _pack_x(xt):
    # xt [D, L] f32 (already scaled): -> [128, NXS*2*NDT*XS] fp8 hi|lo
    NXS, XS, NDT, P = 4, 512, 8, 128
    v = xt.reshape(NDT, P, NXS, XS).transpose(1, 2, 0, 3)  # p s d c
    h, l = _hi_lo(v)
    out = np.stack([h, l], axis=2)  # p s z d c
    return np.ascontiguousarray(out.reshape(P, NXS * 2 * NDT * XS))


def _pack_w_mmajor(w):
    # w [K, N] f32 scaled -> [128, NIT*2*NDT*128] fp8, m-major hi|lo
    K, N = w.shape
    v = (w.reshape(K // 128, 128, N // 128, 128)   # d p m c
         .transpose(1, 2, 0, 3))                   # p m d c
    h, l = _hi_lo(v)
    out = np.stack([h, l], axis=2)                 # p m z d c
    return np.ascontiguousarray(out.reshape(128, (N // 128) * 2 * (K // 128) * 128))


def _pack_wv(w):
    # w [K, N] f32 scaled -> [128, 2*NDT*N] fp8 hi|lo
    K, N = w.shape
    v = w.reshape(K // 128, 128, N).transpose(1, 0, 2)  # p d c
    h, l = _hi_lo(v)
    out = np.stack([h, l], axis=1)                      # p z d c
    return np.ascontiguousarray(out.reshape(128, 2 * (K // 128) * N))


def _pack_wo(w):
    import ml_dtypes
    K, N = w.shape
    v = w.reshape(K // 128, 128, N).transpose(1, 0, 2).reshape(128, (K // 128) * N)
    return np.ascontiguousarray(v).astype(ml_dtypes.bfloat16)


def _make_in_maps(x1, x2, Wq, Wkv, Wo):
    x1h = [_pack_x(x1[b].T * XSC) for b in range(B)]
    x2h = [_pack_x(x2[b].T * XSC) for b in range(B)]
    in_maps = []
    for c in range(N_CORES):
        b, t = c // 2, c % 2
        in_maps.append({
            "x1h": x1h[b],
            "x2h": x2h[b],
            "wqh": _pack_w_mmajor(Wq[:, t * IH:(t + 1) * IH] * WSC),
            "wkh": _pack_w_mmajor(Wkv[:, t * IH:(t + 1) * IH] * WSC),
            "wvh": _pack_wv(Wkv[:, INNER + t * IH: INNER + (t + 1) * IH] * WSC),
            "woh": _pack_wo(Wo[t * IH:(t + 1) * IH, :]),
        })
    return in_maps


def kernel(x1, x2, Wq, Wkv, Wo, bo):
    import sys
    if "/opt/trn_rl_repo" not in sys.path:
        sys.path.insert(0, "/opt/trn_rl_repo")
    from concourse.bass_utils import run_bass_kernel_spmd

    x1 = np.asarray(x1, dtype=np.float32)
    x2 = np.asarray(x2, dtype=np.float32)
    Wq = np.asarray(Wq, dtype=np.float32)
    Wkv = np.asarray(Wkv, dtype=np.float32)
    Wo = np.asarray(Wo, dtype=np.float32)
    bo = np.asarray(bo, dtype=np.float32)

    nc = _get_nc()
    res = run_bass_kernel_spmd(nc, _make_in_maps(x1, x2, Wq, Wkv, Wo),
                               list(range(N_CORES)))
    y = np.empty((B, L, D), dtype=np.float32)
    for b in range(B):
        y[b] = (res.results[2 * b]["yt"] + res.results[2 * b + 1]["yt"]).T + bo
    return y


# revision 3
# speedup vs baseline: 1.0013x; 1.0013x over previous
"""Trainium2 Bass kernel for CrossAttention (B=4, L=2048, D=1024, 16 heads x 64).

Sharding: 8 cores = 4 batches x 2 head-halves (tensor parallel: Wq/Wkv
column-split, Wo row-split).  Host packs x (*8) and W (*32) as fp8-e4m3
hi/lo pairs; the 256x output scale cancels via the softmax denominator
(ones column = 256) and the exp scale (SCALE/65536).

v7 vs v6 (349288 -> 321482 ns, rel err 3.8e-3):
- K/Q/V projections as fp8 DoubleRow 3-term jobs (wh*xh + wh*xl + wl*xh,
  12 DR matmuls at 0.5 cyc/row): 196.6k -> 147.5k PE rows, ~bf16 quality
  (hi/lo split reconstructs bf16; the dropped lo*lo term is ~2^-8).
- U^T restructure: O^T[q,d] = sum_j E_j^T V_j with q on partitions; out
  free size 65 (64 v-dims + denominator column riding as V col 64), so
  U drops 262k -> 135k rows.  Per-head PSUM accumulator [128, 4qt x 65]
  (one bank; 4 sub-accumulation groups share the bank's zero region, one
  start/stop pair).  Normalization: stage u->SBUF (frees the bank fast),
  reciprocal of the denominator strip + 8 tensor_scalar muls -> O bf16,
  then PE transposes (64-row, via identity) -> O^T for the Y matmuls;
  transposes deferred to slot j==9 of the next pair so the in-order PE
  never waits on the DVE normalization chain.
- S stays bf16 (fp8 would cost ~3.4e-2 rel err).  S j-tiles allocate
  per-j [128,1024] pool tiles (bufs=2): a single shared multi-bank tile
  serializes S(j+1) behind exp(j) in the scheduler and costs +150us.
- Engine busy: PE 261us (vs 327.7 in v6), ACT 267us (the 256-exp stream
  is the pacer for blocks 1-3), DVE ~115us.  Block 0 is PE-bound (the 36
  K/V/Q jobs must precede first use), costing ~20us of ACT idle; startup
  ~12us is DMA-paced (fp8 halves the weight/x bytes vs v6).

PSUM: stj 2x[128,1024] (4 banks) + u 2x[128,260] + jobs 2x[128,512] = 8.
Tail: 4 Y-partials on freed st slots + 2 on jobs slots pre-accumulate
pairs 0-2 while pair 3 drains; final transposes ride the idle u banks.
"""

import numpy as np

B = 4
L = 2048
D = 1024
INNER = 1024
HEADS = 16
DH = 64
N_CORES = 8
IH = INNER // 2
SCALE = DH ** -0.5
XSC = 8.0    # host x scale
WSC = 32.0   # host W scale
PS = XSC * WSC  # projection output scale (256)

_CACHE = {}


def _build_nc():
    import os
    VAR = int(os.environ.get("V7_VARIANT", "0"))
    import concourse.bass as bass
    import concourse.tile as tile
    from concourse import bacc, mybir
    from concourse.masks import make_identity

    f32 = mybir.dt.float32
    bf16 = mybir.dt.bfloat16
    fp8 = mybir.dt.float8e4
    DR = mybir.MatmulPerfMode.DoubleRow
    P = 128
    L_, D_, IH_, DH_ = L, D, IH, DH
    LQB = 512
    XS = 512
    NH = IH_ // DH_        # 8 heads per core
    NPAIR = NH // 2        # 4
    NJT = L_ // P          # 16 j tiles
    NQT = LQB // P         # 4 q subtiles per block
    NLQB = L_ // LQB       # 4
    NDT = D_ // P          # 8
    NIT = IH_ // P         # 4 inner tiles (pairs)
    NXS = L_ // XS         # 4
    VW = DH_ + 1           # 65: v + denominator column

    nc = bacc.Bacc("TRN2", target_bir_lowering=False, debug=False)
    # x: [p, s*2*NDT*XS + hl*NDT*XS + d*XS + c] fp8 (hi|lo per slice)
    x1 = nc.declare_dram_parameter("x1h", [P, NXS * 2 * NDT * XS], fp8,
                                   isOutput=False)
    x2 = nc.declare_dram_parameter("x2h", [P, NXS * 2 * NDT * XS], fp8,
                                   isOutput=False)
    # wq/wk m-major: [p, m*2*NDT*P + hl*NDT*P + d*P + c] fp8
    wq = nc.declare_dram_parameter("wqh", [P, NIT * 2 * NDT * P], fp8,
                                   isOutput=False)
    wk = nc.declare_dram_parameter("wkh", [P, NIT * 2 * NDT * P], fp8,
                                   isOutput=False)
    # wv: [p, hl*NDT*IH + d*IH + c] fp8
    wv = nc.declare_dram_parameter("wvh", [P, 2 * NDT * IH_], fp8,
                                   isOutput=False)
    wo = nc.declare_dram_parameter("woh", [P, NIT * D_], bf16, isOutput=False)
    yt = nc.declare_dram_parameter("yt", [D_, L_], f32, isOutput=True)

    with tile.TileContext(nc) as tc:
        with tc.tile_pool(name="persist", bufs=1) as persist:
            kt = persist.tile([P, NIT * L_], bf16, name="kt")
            qt = persist.tile([P, NIT * L_], bf16, name="qt")
            vva = persist.tile([P, NJT * NH * VW], bf16, name="vva")
            ident = persist.tile([P, P], bf16, name="ident")
            wkt = persist.tile([P, NIT * 2 * NDT * P], fp8, name="wkt")
            wqt = persist.tile([P, NIT * 2 * NDT * P], fp8, name="wqt")
            wvt = persist.tile([P, 2 * NDT * IH_], fp8, name="wvt")
            wot = persist.tile([P, NIT * D_], bf16, name="wot")
            make_identity(nc, ident)
            # denominator ride-along columns (col 64 of each [.,j,h,65])
            vvav = vva.rearrange("p (j h c) -> p j h c", h=NH, c=VW)
            nc.vector.memset(vvav[:, :, :, DH_:], float(PS))

            with (
                tc.tile_pool(name="xts2", bufs=1) as xts2,
                tc.tile_pool(name="xts1", bufs=2) as xts1,
                tc.tile_pool(name="stj", bufs=2, space="PSUM") as stjp,
                tc.tile_pool(name="upool", bufs=2, space="PSUM") as upool,
                tc.tile_pool(name="jobs", bufs=2, space="PSUM") as jobs,
                tc.tile_pool(name="ets", bufs=8) as ets,
                tc.tile_pool(name="smalls", bufs=2) as smalls,
                tc.tile_pool(name="osbp", bufs=4) as osbp,
                tc.tile_pool(name="otsb", bufs=8) as otsbp,
                tc.tile_pool(name="youts", bufs=6) as youts,
            ):
                st = None

                # ---- DMA: deadline-ordered, hi-halves first ----
                xt2, xt1s = {}, {}
                XB = 2 * NDT * XS          # per-slice block (8192)
                HB = NDT * XS              # hi/lo half (4096)
                WB = 2 * NDT * P           # per-m block (2048)
                WH = NDT * P               # 1024

                def load_x2_slice(s):
                    t = xts2.tile([P, XB], fp8, name="x2t", tag=f"x2_{s}",
                                  bufs=1)
                    xt2[s] = t
                    nc.sync.dma_start(out=t, in_=x2[:, s * XB:(s + 1) * XB])

                def load_x1_slice(sl):
                    t = xts1.tile([P, XB], fp8, name="x1t", tag="x1", bufs=2)
                    xt1s[sl] = t
                    nc.sync.dma_start(out=t, in_=x1[:, sl * XB:(sl + 1) * XB])

                nc.sync.dma_start(out=wkt[:, 0:WB], in_=wk[:, 0:WB])
                nc.sync.dma_start(out=wqt[:, 0:WB], in_=wq[:, 0:WB])
                t0 = xts2.tile([P, XB], fp8, name="x2t", tag="x2_0", bufs=1)
                xt2[0] = t0
                nc.sync.dma_start(out=t0[:, 0:HB], in_=x2[:, 0:HB])
                nc.sync.dma_start(out=t0[:, HB:], in_=x2[:, HB:XB])
                t1 = xts1.tile([P, XB], fp8, name="x1t", tag="x1", bufs=2)
                xt1s[0] = t1
                nc.sync.dma_start(out=t1[:, 0:HB], in_=x1[:, 0:HB])
                nc.sync.dma_start(out=t1[:, HB:], in_=x1[:, HB:XB])
                nc.sync.dma_start(out=wvt[:, 0:HB], in_=wv[:, 0:HB])
                nc.sync.dma_start(out=wvt[:, HB:], in_=wv[:, HB:])
                load_x2_slice(1)
                load_x2_slice(2)
                load_x2_slice(3)
                nc.sync.dma_start(out=wqt[:, WB:2 * WB], in_=wq[:, WB:2 * WB])
                nc.sync.dma_start(out=wkt[:, WB:2 * WB], in_=wk[:, WB:2 * WB])
                nc.sync.dma_start(out=wqt[:, 2 * WB:], in_=wq[:, 2 * WB:])
                nc.sync.dma_start(out=wkt[:, 2 * WB:], in_=wk[:, 2 * WB:])
                nc.sync.dma_start(out=wot, in_=wo[:, :])

                # ---- fp8 DR 3-term projection jobs ----
                def kjob(m, s):
                    t = jobs.tile([P, XS], f32, name="kj", tag="job")
                    wv_ = wkt.rearrange("p (m z d c) -> p m z d c", m=NIT,
                                        z=2, c=P)
                    xv_ = xt2[s].rearrange("p (z d c) -> p z d c", z=2, c=XS)
                    n = 0
                    for whl, xhl in ((0, 0), (0, 1), (1, 0)):
                        for d in range(0, NDT, 2):
                            nc.tensor.matmul(
                                t, lhsT=wv_[:, m, whl, d:d + 2, :],
                                rhs=xv_[:, xhl, d:d + 2, :],
                                start=(n == 0), stop=(n == 11), perf_mode=DR)
                            n += 1
                    nc.vector.tensor_copy(
                        kt[:, m * L_ + s * XS: m * L_ + (s + 1) * XS], t)

                def qjob(sl, m):
                    t = jobs.tile([P, XS], f32, name="qj", tag="job")
                    wv_ = wqt.rearrange("p (m z d c) -> p m z d c", m=NIT,
                                        z=2, c=P)
                    xv_ = xt1s[sl].rearrange("p (z d c) -> p z d c", z=2, c=XS)
                    n = 0
                    for whl, xhl in ((0, 0), (0, 1), (1, 0)):
                        for d in range(0, NDT, 2):
                            nc.tensor.matmul(
                                t, lhsT=wv_[:, m, whl, d:d + 2, :],
                                rhs=xv_[:, xhl, d:d + 2, :],
                                start=(n == 0), stop=(n == 11), perf_mode=DR)
                            n += 1
                    nc.vector.tensor_copy(
                        qt[:, m * L_ + sl * XS: m * L_ + (sl + 1) * XS], t)

                def vjob(s, tq):
                    t = jobs.tile([P, IH_], f32, name="vj", tag="job")
                    xv_ = xt2[s].rearrange("p (z d c) -> p z d c", z=2, c=XS)
                    wv_ = wvt.rearrange("p (z d c) -> p z d c", z=2, c=IH_)
                    n = 0
                    for xhl, whl in ((0, 0), (1, 0), (0, 1)):
                        for d in range(0, NDT, 2):
                            nc.tensor.matmul(
                                t,
                                lhsT=xv_[:, xhl, d:d + 2, tq * P:(tq + 1) * P],
                                rhs=wv_[:, whl, d:d + 2, :],
                                start=(n == 0), stop=(n == 11), perf_mode=DR)
                            n += 1
                    j = s * (XS // P) + tq
                    dst = (vva[:, j * NH * VW:(j + 1) * NH * VW]
                           .rearrange("p (h c) -> p h c", c=VW)[:, :, :DH_])
                    nc.vector.tensor_copy(
                        dst, t.rearrange("p (h c) -> p h c", c=DH_))

                def run_job(spec):
                    kind = spec[0]
                    if kind == "k":
                        kjob(spec[1], spec[2])
                    elif kind == "v":
                        vjob(spec[1], spec[2])
                    elif kind == "q":
                        qjob(spec[1], spec[2])

                # ---- attention ----
                ROT = [(0, 1), (2, 3)]  # st quarter pairs, ping-pong by jg

                def emit_s(hp, j, lqb, jg):
                    stt = stjp.tile([P, 2 * LQB], f32, name="stt", tag="st")
                    base = 0
                    for hh in range(2):
                        nc.tensor.matmul(
                            stt[:, base + hh * LQB: base + (hh + 1) * LQB],
                            lhsT=kt[hh * DH_:(hh + 1) * DH_,
                                    hp * L_ + j * P: hp * L_ + (j + 1) * P],
                            rhs=qt[hh * DH_:(hh + 1) * DH_,
                                   hp * L_ + lqb * LQB:
                                   hp * L_ + lqb * LQB + LQB],
                            start=True, stop=True)
                    et = ets.tile([P, 2 * LQB], bf16, name="et")
                    nc.scalar.activation(
                        et, stt[:, base: base + 2 * LQB],
                        mybir.ActivationFunctionType.Exp,
                        scale=float(SCALE / (PS * PS)))
                    return et

                def emit_u(hp, et, j, u0, u1):
                    if VAR >= 4:
                        return
                    for hh, u in ((0, u0), (1, u1)):
                        h = 2 * hp + hh
                        for qn in range(NQT):
                            nc.tensor.matmul(
                                u[:, qn * VW:(qn + 1) * VW],
                                lhsT=et[:, hh * LQB + qn * P:
                                        hh * LQB + (qn + 1) * P],
                                rhs=vva[:, (j * NH + h) * VW:
                                        (j * NH + h + 1) * VW],
                                start=(j == 0 and qn == 0),
                                stop=(j == NJT - 1 and qn == NQT - 1))

                def drain_pair(hp, u0, u1, lqb, otsb_cur):
                    if VAR >= 2:
                        return
                    # stage u -> SBUF fast (frees the psum banks), then
                    # normalize from the staged copy
                    us = smalls.tile([P, 2 * NQT * VW], f32, name="us",
                                     tag="us", bufs=2)
                    nc.vector.tensor_copy(us[:, 0:NQT * VW], u0)
                    nc.vector.tensor_copy(us[:, NQT * VW:], u1)
                    rcp = smalls.tile([P, 2 * NQT], f32, name="rcp",
                                      tag="rcp", bufs=2)
                    usv = us.rearrange("p (q c) -> p q c", c=VW)
                    nc.vector.reciprocal(
                        rcp, usv[:, :, DH_])
                    osbs = []
                    for qn in range(NQT):
                        osb = osbp.tile([P, 2 * DH_], bf16, name="osb")
                        osbs.append(osb)
                        nc.vector.tensor_scalar_mul(
                            osb[:, 0:DH_], usv[:, qn, 0:DH_],
                            rcp[:, qn:qn + 1])
                        nc.vector.tensor_scalar_mul(
                            osb[:, DH_:], usv[:, NQT + qn, 0:DH_],
                            rcp[:, NQT + qn:NQT + qn + 1])
                    if VAR >= 1:
                        return
                    tdefer.append((osbs, otsb_cur[hp]))

                def flush_transposes(pool=None):
                    if not tdefer:
                        return
                    pool = pool or jobs
                    osbs, ot = tdefer.pop(0)
                    for qn in range(NQT):
                        for hh in range(2):
                            tp = pool.tile([DH_, P], bf16, name="tp",
                                           tag="job" if pool is jobs else "u")
                            nc.tensor.matmul(
                                tp, lhsT=osbs[qn][:, hh * DH_:(hh + 1) * DH_],
                                rhs=ident, is_transpose=True)
                            nc.vector.tensor_copy(
                                ot[hh * DH_:(hh + 1) * DH_,
                                   qn * P:(qn + 1) * P], tp)

                def yjob(otsb_prev, lqb_prev, dsub, py=None, it0=0,
                         halves=False):
                    if VAR >= 3:
                        return
                    if py is None:
                        py = jobs.tile([P, LQB], f32, name="py", tag="job")
                    for it in range(it0, NIT):
                        nc.tensor.matmul(
                            py[:, 0:LQB],
                            lhsT=wot[:, it * D_ + dsub * P:
                                     it * D_ + (dsub + 1) * P],
                            rhs=otsb_prev[it],
                            start=(it == 0), stop=(it == NIT - 1))
                    yo = youts.tile([P, LQB], f32, name="yo")
                    nh = 2 if halves else 1
                    hw_ = LQB // nh
                    for hh in range(nh):
                        nc.vector.tensor_copy(
                            yo[:, hh * hw_:(hh + 1) * hw_],
                            py[:, hh * hw_:(hh + 1) * hw_])
                        nc.sync.dma_start(
                            out=yt[dsub * P:(dsub + 1) * P,
                                   lqb_prev * LQB + hh * hw_:
                                   lqb_prev * LQB + (hh + 1) * hw_],
                            in_=yo[:, hh * hw_:(hh + 1) * hw_])

                def yjob_partial(otsb_prev, dsub, pool_tile):
                    if VAR >= 3:
                        return
                    for it in range(NIT - 1):
                        nc.tensor.matmul(
                            pool_tile[:, 0:LQB],
                            lhsT=wot[:, it * D_ + dsub * P:
                                     it * D_ + (dsub + 1) * P],
                            rhs=otsb_prev[it],
                            start=(it == 0), stop=False)

                # block-0 job inserts at (pair, j) slots
                b0_ins = {
                    (0, 0): [("v", 0, 0)], (0, 1): [("v", 0, 1)],
                    (0, 2): [("v", 0, 2)], (0, 3): [("v", 0, 3), ("k", 0, 1)],
                    (0, 4): [("v", 1, 0)], (0, 5): [("v", 1, 1)],
                    (0, 6): [("v", 1, 2), ("k", 0, 2)], (0, 7): [("v", 1, 3)],
                    (0, 8): [("v", 2, 0)], (0, 9): [("v", 2, 1)],
                    (0, 10): [("v", 2, 2), ("k", 0, 3)], (0, 11): [("v", 2, 3)],
                    (0, 12): [("v", 3, 0)], (0, 13): [("v", 3, 1)],
                    (0, 14): [("v", 3, 2), ("q", 0, 1)],
                    (0, 15): [("v", 3, 3), ("k", 1, 0)],
                    (1, 0): [("k", 1, 1)], (1, 2): [("k", 1, 2)],
                    (1, 4): [("k", 1, 3)], (1, 6): [("k", 2, 0)],
                    (1, 8): [("k", 2, 1)], (1, 10): [("k", 2, 2)],
                    (1, 12): [("k", 2, 3)], (1, 14): [("q", 0, 2)],
                    (2, 0): [("k", 3, 0)], (2, 2): [("k", 3, 1)],
                    (2, 4): [("k", 3, 2)], (2, 6): [("k", 3, 3)],
                    (2, 8): [("q", 0, 3)], (2, 12): [("q", 1, 0)],
                    (3, 2): [("q", 1, 1)], (3, 6): [("q", 1, 2)],
                    (3, 10): [("q", 1, 3)],
                }


                # prologue
                kjob(0, 0)
                qjob(0, 0)

                jg = 0
                tdefer = []    # deferred transpose batches
                pend = []      # (hp, et, j, u0, u1, lqb, otsb_cur)
                drains = []    # pairs awaiting drain after last U pop
                ydefer = []
                otsb_blocks = []  # per lqb: dict pair -> ot tile
                partials = []

                for lqb in range(NLQB):
                    if lqb < NLQB - 1:
                        load_x1_slice(lqb + 1)
                    otsb_cur = {hp: otsbp.tile([P, LQB], bf16, name="ot")
                                for hp in range(NPAIR)}
                    otsb_blocks.append(otsb_cur)
                    for hp in range(NPAIR):
                        u0 = upool.tile([P, NQT * VW], f32, name="u0",
                                        tag="u")
                        u1 = upool.tile([P, NQT * VW], f32, name="u1",
                                        tag="u")
                        for j in range(NJT):
                            et = emit_s(hp, j, lqb, jg)
                            jg += 1
                            pend.append((hp, et, j, u0, u1, lqb, otsb_cur))
                            while len(pend) > 4:
                                e = pend.pop(0)
                                emit_u(e[0], e[1], e[2], e[3], e[4])
                                if e[2] == NJT - 1:
                                    drain_pair(e[0], e[3], e[4], e[5], e[6])
                            if j == 9:
                                flush_transposes()
                            if (lqb == NLQB - 1 and hp == NPAIR - 1
                                    and j in (12, 14) and VAR == 0):
                                ds = 4 + (j - 12) // 2
                                pj = jobs.tile([P, LQB], f32, name="py",
                                               tag="job")
                                yjob_partial(
                                    [otsb_blocks[-1][i] for i in range(3)]
                                    + [None], ds, pj)
                                partials.append((ds, pj))
                            if VAR >= 5:
                                pass
                            elif lqb == 0:
                                for spec in b0_ins.get((hp, j), ()):
                                    run_job(spec)
                            else:
                                if j in (10, 13) and ydefer:
                                    yjob(*ydefer.pop(0))
                                if j == 8 and lqb < NLQB - 1:
                                    qjob(lqb + 1, hp)
                        if lqb == NLQB - 1 and hp == NPAIR - 1:
                            # flush: last pair's U + drain
                            while pend:
                                e = pend.pop(0)
                                emit_u(e[0], e[1], e[2], e[3], e[4])
                                if e[2] == NJT - 1:
                                    if e[0] == NPAIR - 1:
                                        # partials on freed st slots
                                        prev = otsb_blocks[-1]
                                        pprev = [prev[i] for i in range(3)]
                                        for sl in range(2):
                                            big = stjp.tile(
                                                [P, 2 * LQB], f32,
                                                name="stt", tag="st")
                                            for hh2 in range(2):
                                                ds = sl * 2 + hh2
                                                pt = big[:, hh2 * LQB:
                                                         (hh2 + 1) * LQB]
                                                yjob_partial(pprev + [None],
                                                             ds, pt)
                                                partials.append((ds, pt))
                                    drain_pair(e[0], e[3], e[4], e[5], e[6])
                    skip = ()
                    if lqb == NLQB - 1:
                        skip = tuple(range(D_ // P))
                    ydefer.extend(
                        ([otsb_blocks[lqb][i] for i in range(NPAIR)], lqb, ds)
                        for ds in range(D_ // P) if ds not in skip)

                while tdefer:
                    flush_transposes(upool)
                # tail: finish partials (it=3) then remaining yjobs
                last = [otsb_blocks[-1][i] for i in range(NPAIR)]
                for ds, pt in partials:
                    yjob(last, NLQB - 1, ds, py=pt, it0=NIT - 1)
                for ds in range(6, D_ // P):
                    yjob(last, NLQB - 1, ds)
                while ydefer:
                    yjob(*ydefer.pop(0))
    nc.compile()
    return nc


def _get_nc():
    if "nc" not in _CACHE:
        _CACHE["nc"] = _build_nc()
    return _CACHE["nc"]


def _f8():
    import ml_dtypes
    return ml_dtypes.float8_e4m3


def _hi_lo(a):
    f8 = _f8()
    h = a.astype(f8)
    l = (a - h.astype(np.float32)).astype(f8)
    return h, l


def _pack_x(xt):
    # xt [D, L] f32 (already scaled): -> [128, NXS*2*NDT*XS] fp8 hi|lo
    NXS, XS, NDT, P = 4, 512, 8, 128
    v = xt.reshape(NDT, P, NXS, XS).transpose(1, 2, 0, 3)  # p s d c
    h, l = _hi_lo(v)
    out = np.stack([h, l], axis=2)  # p s z d c
    return np.ascontiguousarray(out.reshape(P, NXS * 2 * NDT * XS))


def _pack_w_mmajor(w):
    # w [K, N] f32 scaled -> [128, NIT*2*NDT*128] fp8, m-major hi|lo
    K, N = w.shape
    v = (w.reshape(K // 128, 128, N // 128, 128)   # d p m c
         .transpose(1, 2, 0, 3))                   # p m d c
    h, l = _hi_lo(v)
    out = np.stack([h, l], axis=2)                 # p m z d c
    return np.ascontiguousarray(out.reshape(128, (N // 128) * 2 * (K // 128) * 128))


def _pack_wv(w):
    # w [K, N] f32 scaled -> [128, 2*NDT*N] fp8 hi|lo
    K, N = w.shape
    v = w.reshape(K // 128, 128, N).transpose(1, 0, 2)  # p d c
    h, l = _hi_lo(v)
    out = np.stack([h, l], axis=1)                      # p z d c
    return np.ascontiguousarray(out.reshape(128, 2 * (K // 128) * N))


def _pack_wo(w):
    import ml_dtypes
    K, N = w.shape
    v = w.reshape(K // 128, 128, N).transpose(1, 0, 2).reshape(128, (K // 128) * N)
    return np.ascontiguousarray(v).astype(ml_dtypes.bfloat16)


def _make_in_maps(x1, x2, Wq, Wkv, Wo):
    x1h = [_pack_x(x1[b].T * XSC) for b in range(B)]
    x2h = [_pack_x(x2[b].T * XSC) for b in range(B)]
    in_maps = []
    for c in range(N_CORES):
        b, t = c // 2, c % 2
        in_maps.append({
            "x1h": x1h[b],
            "x2h": x2h[b],
            "wqh": _pack_w_mmajor(Wq[:, t * IH:(t + 1) * IH] * WSC),
            "wkh": _pack_w_mmajor(Wkv[:, t * IH:(t + 1) * IH] * WSC),
            "wvh": _pack_wv(Wkv[:, INNER + t * IH: INNER + (t + 1) * IH] * WSC),
            "woh": _pack_wo(Wo[t * IH:(t + 1) * IH, :]),
        })
    return in_maps


def kernel(x1, x2, Wq, Wkv, Wo, bo):
    import sys
    if "/opt/trn_rl_repo" not in sys.path:
        sys.path.insert(0, "/opt/trn_rl_repo")
    from concourse.bass_utils import run_bass_kernel_spmd

    x1 = np.asarray(x1, dtype=np.float32)
    x2 = np.asarray(x2, dtype=np.float32)
    Wq = np.asarray(Wq, dtype=np.float32)
    Wkv = np.asarray(Wkv, dtype=np.float32)
    Wo = np.asarray(Wo, dtype=np.float32)
    bo = np.asarray(bo, dtype=np.float32)

    nc = _get_nc()
    res = run_bass_kernel_spmd(nc, _make_in_maps(x1, x2, Wq, Wkv, Wo),
                               list(range(N_CORES)))
    y = np.empty((B, L, D), dtype=np.float32)
    for b in range(B):
        y[b] = (res.results[2 * b]["yt"] + res.results[2 * b + 1]["yt"]).T + bo
    return y


# revision 4
# speedup vs baseline: 1.0016x; 1.0003x over previous
"""Trainium2 Bass kernel for CrossAttention (B=4, L=2048, D=1024, 16 heads x 64).

Sharding: 8 cores = 4 batches x 2 head-halves (tensor parallel: Wq/Wkv
column-split, Wo row-split).  Host packs x (*8) and W (*32) as fp8-e4m3
hi/lo pairs; the 256x output scale cancels via the softmax denominator
(ones column = 256) and the exp scale (SCALE/65536).

v7 vs v6 (349288 -> 321060 ns, rel err 3.8e-3):
- K/Q/V projections as fp8 DoubleRow 3-term jobs (wh*xh + wh*xl + wl*xh,
  12 DR matmuls at 0.5 cyc/row): 196.6k -> 147.5k PE rows, ~bf16 quality
  (hi/lo split reconstructs bf16; the dropped lo*lo term is ~2^-8).
- U^T restructure: O^T[q,d] = sum_j E_j^T V_j with q on partitions; out
  free size 65 (64 v-dims + denominator column riding as V col 64), so
  U drops 262k -> 135k rows.  Per-head PSUM accumulator [128, 4qt x 65]
  (one bank; 4 sub-accumulation groups share the bank's zero region, one
  start/stop pair).  Normalization: stage u->SBUF (frees the bank fast),
  reciprocal of the denominator strip + 8 tensor_scalar muls -> O bf16,
  then PE transposes (64-row, via identity) -> O^T for the Y matmuls;
  transposes deferred to slot j==9 of the next pair so the in-order PE
  never waits on the DVE normalization chain.
- S stays bf16 (fp8 would cost ~3.4e-2 rel err).  S j-tiles allocate
  per-j [128,1024] pool tiles (bufs=2): a single shared multi-bank tile
  serializes S(j+1) behind exp(j) in the scheduler and costs +150us.
- Engine busy: PE 261us (vs 327.7 in v6), ACT 267us (the 256-exp stream
  is the pacer for blocks 1-3), DVE ~115us.  Block 0 is PE-bound (the 36
  K/V/Q jobs must precede first use), costing ~20us of ACT idle; startup
  ~12us is DMA-paced (fp8 halves the weight/x bytes vs v6).

PSUM: stj 2x[128,1024] (4 banks) + u 2x[128,260] + jobs 2x[128,512] = 8.
Tail: 4 Y-partials on freed st slots + 2 on jobs slots pre-accumulate
pairs 0-2 while pair 3 drains; final transposes ride the idle u banks and
tail evictions alternate ACT (idle after the last exp; scalar Copy reads
PSUM) with DVE to halve the tail's serial eviction chain.
"""

import numpy as np

B = 4
L = 2048
D = 1024
INNER = 1024
HEADS = 16
DH = 64
N_CORES = 8
IH = INNER // 2
SCALE = DH ** -0.5
XSC = 8.0    # host x scale
WSC = 32.0   # host W scale
PS = XSC * WSC  # projection output scale (256)

_CACHE = {}


def _build_nc():
    import os
    VAR = int(os.environ.get("V7_VARIANT", "0"))
    import concourse.bass as bass
    import concourse.tile as tile
    from concourse import bacc, mybir
    from concourse.masks import make_identity

    f32 = mybir.dt.float32
    bf16 = mybir.dt.bfloat16
    fp8 = mybir.dt.float8e4
    DR = mybir.MatmulPerfMode.DoubleRow
    P = 128
    L_, D_, IH_, DH_ = L, D, IH, DH
    LQB = 512
    XS = 512
    NH = IH_ // DH_        # 8 heads per core
    NPAIR = NH // 2        # 4
    NJT = L_ // P          # 16 j tiles
    NQT = LQB // P         # 4 q subtiles per block
    NLQB = L_ // LQB       # 4
    NDT = D_ // P          # 8
    NIT = IH_ // P         # 4 inner tiles (pairs)
    NXS = L_ // XS         # 4
    VW = DH_ + 1           # 65: v + denominator column

    nc = bacc.Bacc("TRN2", target_bir_lowering=False, debug=False)
    # x: [p, s*2*NDT*XS + hl*NDT*XS + d*XS + c] fp8 (hi|lo per slice)
    x1 = nc.declare_dram_parameter("x1h", [P, NXS * 2 * NDT * XS], fp8,
                                   isOutput=False)
    x2 = nc.declare_dram_parameter("x2h", [P, NXS * 2 * NDT * XS], fp8,
                                   isOutput=False)
    # wq/wk m-major: [p, m*2*NDT*P + hl*NDT*P + d*P + c] fp8
    wq = nc.declare_dram_parameter("wqh", [P, NIT * 2 * NDT * P], fp8,
                                   isOutput=False)
    wk = nc.declare_dram_parameter("wkh", [P, NIT * 2 * NDT * P], fp8,
                                   isOutput=False)
    # wv: [p, hl*NDT*IH + d*IH + c] fp8
    wv = nc.declare_dram_parameter("wvh", [P, 2 * NDT * IH_], fp8,
                                   isOutput=False)
    wo = nc.declare_dram_parameter("woh", [P, NIT * D_], bf16, isOutput=False)
    yt = nc.declare_dram_parameter("yt", [D_, L_], f32, isOutput=True)

    with tile.TileContext(nc) as tc:
        with tc.tile_pool(name="persist", bufs=1) as persist:
            kt = persist.tile([P, NIT * L_], bf16, name="kt")
            qt = persist.tile([P, NIT * L_], bf16, name="qt")
            vva = persist.tile([P, NJT * NH * VW], bf16, name="vva")
            ident = persist.tile([P, P], bf16, name="ident")
            wkt = persist.tile([P, NIT * 2 * NDT * P], fp8, name="wkt")
            wqt = persist.tile([P, NIT * 2 * NDT * P], fp8, name="wqt")
            wvt = persist.tile([P, 2 * NDT * IH_], fp8, name="wvt")
            wot = persist.tile([P, NIT * D_], bf16, name="wot")
            make_identity(nc, ident)
            # denominator ride-along columns (col 64 of each [.,j,h,65])
            vvav = vva.rearrange("p (j h c) -> p j h c", h=NH, c=VW)
            nc.vector.memset(vvav[:, :, :, DH_:], float(PS))

            with (
                tc.tile_pool(name="xts2", bufs=1) as xts2,
                tc.tile_pool(name="xts1", bufs=2) as xts1,
                tc.tile_pool(name="stj", bufs=2, space="PSUM") as stjp,
                tc.tile_pool(name="upool", bufs=2, space="PSUM") as upool,
                tc.tile_pool(name="jobs", bufs=2, space="PSUM") as jobs,
                tc.tile_pool(name="ets", bufs=8) as ets,
                tc.tile_pool(name="smalls", bufs=2) as smalls,
                tc.tile_pool(name="osbp", bufs=4) as osbp,
                tc.tile_pool(name="otsb", bufs=8) as otsbp,
                tc.tile_pool(name="youts", bufs=6) as youts,
            ):
                st = None

                # ---- DMA: deadline-ordered, hi-halves first ----
                xt2, xt1s = {}, {}
                XB = 2 * NDT * XS          # per-slice block (8192)
                HB = NDT * XS              # hi/lo half (4096)
                WB = 2 * NDT * P           # per-m block (2048)
                WH = NDT * P               # 1024

                def load_x2_slice(s):
                    t = xts2.tile([P, XB], fp8, name="x2t", tag=f"x2_{s}",
                                  bufs=1)
                    xt2[s] = t
                    nc.sync.dma_start(out=t, in_=x2[:, s * XB:(s + 1) * XB])

                def load_x1_slice(sl):
                    t = xts1.tile([P, XB], fp8, name="x1t", tag="x1", bufs=2)
                    xt1s[sl] = t
                    nc.sync.dma_start(out=t, in_=x1[:, sl * XB:(sl + 1) * XB])

                nc.sync.dma_start(out=wkt[:, 0:WB], in_=wk[:, 0:WB])
                nc.sync.dma_start(out=wqt[:, 0:WB], in_=wq[:, 0:WB])
                t0 = xts2.tile([P, XB], fp8, name="x2t", tag="x2_0", bufs=1)
                xt2[0] = t0
                nc.sync.dma_start(out=t0[:, 0:HB], in_=x2[:, 0:HB])
                nc.sync.dma_start(out=t0[:, HB:], in_=x2[:, HB:XB])
                t1 = xts1.tile([P, XB], fp8, name="x1t", tag="x1", bufs=2)
                xt1s[0] = t1
                nc.sync.dma_start(out=t1[:, 0:HB], in_=x1[:, 0:HB])
                nc.sync.dma_start(out=t1[:, HB:], in_=x1[:, HB:XB])
                nc.sync.dma_start(out=wvt[:, 0:HB], in_=wv[:, 0:HB])
                nc.sync.dma_start(out=wvt[:, HB:], in_=wv[:, HB:])
                load_x2_slice(1)
                load_x2_slice(2)
                load_x2_slice(3)
                nc.sync.dma_start(out=wqt[:, WB:2 * WB], in_=wq[:, WB:2 * WB])
                nc.sync.dma_start(out=wkt[:, WB:2 * WB], in_=wk[:, WB:2 * WB])
                nc.sync.dma_start(out=wqt[:, 2 * WB:], in_=wq[:, 2 * WB:])
                nc.sync.dma_start(out=wkt[:, 2 * WB:], in_=wk[:, 2 * WB:])
                nc.sync.dma_start(out=wot, in_=wo[:, :])

                # ---- fp8 DR 3-term projection jobs ----
                def kjob(m, s):
                    t = jobs.tile([P, XS], f32, name="kj", tag="job")
                    wv_ = wkt.rearrange("p (m z d c) -> p m z d c", m=NIT,
                                        z=2, c=P)
                    xv_ = xt2[s].rearrange("p (z d c) -> p z d c", z=2, c=XS)
                    n = 0
                    for whl, xhl in ((0, 0), (0, 1), (1, 0)):
                        for d in range(0, NDT, 2):
                            nc.tensor.matmul(
                                t, lhsT=wv_[:, m, whl, d:d + 2, :],
                                rhs=xv_[:, xhl, d:d + 2, :],
                                start=(n == 0), stop=(n == 11), perf_mode=DR)
                            n += 1
                    nc.vector.tensor_copy(
                        kt[:, m * L_ + s * XS: m * L_ + (s + 1) * XS], t)

                def qjob(sl, m):
                    t = jobs.tile([P, XS], f32, name="qj", tag="job")
                    wv_ = wqt.rearrange("p (m z d c) -> p m z d c", m=NIT,
                                        z=2, c=P)
                    xv_ = xt1s[sl].rearrange("p (z d c) -> p z d c", z=2, c=XS)
                    n = 0
                    for whl, xhl in ((0, 0), (0, 1), (1, 0)):
                        for d in range(0, NDT, 2):
                            nc.tensor.matmul(
                                t, lhsT=wv_[:, m, whl, d:d + 2, :],
                                rhs=xv_[:, xhl, d:d + 2, :],
                                start=(n == 0), stop=(n == 11), perf_mode=DR)
                            n += 1
                    nc.vector.tensor_copy(
                        qt[:, m * L_ + sl * XS: m * L_ + (sl + 1) * XS], t)

                def vjob(s, tq):
                    t = jobs.tile([P, IH_], f32, name="vj", tag="job")
                    xv_ = xt2[s].rearrange("p (z d c) -> p z d c", z=2, c=XS)
                    wv_ = wvt.rearrange("p (z d c) -> p z d c", z=2, c=IH_)
                    n = 0
                    for xhl, whl in ((0, 0), (1, 0), (0, 1)):
                        for d in range(0, NDT, 2):
                            nc.tensor.matmul(
                                t,
                                lhsT=xv_[:, xhl, d:d + 2, tq * P:(tq + 1) * P],
                                rhs=wv_[:, whl, d:d + 2, :],
                                start=(n == 0), stop=(n == 11), perf_mode=DR)
                            n += 1
                    j = s * (XS // P) + tq
                    dst = (vva[:, j * NH * VW:(j + 1) * NH * VW]
                           .rearrange("p (h c) -> p h c", c=VW)[:, :, :DH_])
                    nc.vector.tensor_copy(
                        dst, t.rearrange("p (h c) -> p h c", c=DH_))

                def run_job(spec):
                    kind = spec[0]
                    if kind == "k":
                        kjob(spec[1], spec[2])
                    elif kind == "v":
                        vjob(spec[1], spec[2])
                    elif kind == "q":
                        qjob(spec[1], spec[2])

                # ---- attention ----
                ROT = [(0, 1), (2, 3)]  # st quarter pairs, ping-pong by jg

                def emit_s(hp, j, lqb, jg):
                    stt = stjp.tile([P, 2 * LQB], f32, name="stt", tag="st")
                    base = 0
                    for hh in range(2):
                        nc.tensor.matmul(
                            stt[:, base + hh * LQB: base + (hh + 1) * LQB],
                            lhsT=kt[hh * DH_:(hh + 1) * DH_,
                                    hp * L_ + j * P: hp * L_ + (j + 1) * P],
                            rhs=qt[hh * DH_:(hh + 1) * DH_,
                                   hp * L_ + lqb * LQB:
                                   hp * L_ + lqb * LQB + LQB],
                            start=True, stop=True)
                    et = ets.tile([P, 2 * LQB], bf16, name="et")
                    nc.scalar.activation(
                        et, stt[:, base: base + 2 * LQB],
                        mybir.ActivationFunctionType.Exp,
                        scale=float(SCALE / (PS * PS)))
                    return et

                def emit_u(hp, et, j, u0, u1):
                    if VAR >= 4:
                        return
                    for hh, u in ((0, u0), (1, u1)):
                        h = 2 * hp + hh
                        for qn in range(NQT):
                            nc.tensor.matmul(
                                u[:, qn * VW:(qn + 1) * VW],
                                lhsT=et[:, hh * LQB + qn * P:
                                        hh * LQB + (qn + 1) * P],
                                rhs=vva[:, (j * NH + h) * VW:
                                        (j * NH + h + 1) * VW],
                                start=(j == 0 and qn == 0),
                                stop=(j == NJT - 1 and qn == NQT - 1))

                def drain_pair(hp, u0, u1, lqb, otsb_cur):
                    if VAR >= 2:
                        return
                    # stage u -> SBUF fast (frees the psum banks), then
                    # normalize from the staged copy
                    us = smalls.tile([P, 2 * NQT * VW], f32, name="us",
                                     tag="us", bufs=2)
                    nc.vector.tensor_copy(us[:, 0:NQT * VW], u0)
                    nc.vector.tensor_copy(us[:, NQT * VW:], u1)
                    rcp = smalls.tile([P, 2 * NQT], f32, name="rcp",
                                      tag="rcp", bufs=2)
                    usv = us.rearrange("p (q c) -> p q c", c=VW)
                    nc.vector.reciprocal(
                        rcp, usv[:, :, DH_])
                    osbs = []
                    for qn in range(NQT):
                        osb = osbp.tile([P, 2 * DH_], bf16, name="osb")
                        osbs.append(osb)
                        nc.vector.tensor_scalar_mul(
                            osb[:, 0:DH_], usv[:, qn, 0:DH_],
                            rcp[:, qn:qn + 1])
                        nc.vector.tensor_scalar_mul(
                            osb[:, DH_:], usv[:, NQT + qn, 0:DH_],
                            rcp[:, NQT + qn:NQT + qn + 1])
                    if VAR >= 1:
                        return
                    tdefer.append((osbs, otsb_cur[hp]))

                def flush_transposes(pool=None, act_evict=False):
                    if not tdefer:
                        return
                    pool = pool or jobs
                    osbs, ot = tdefer.pop(0)
                    for qn in range(NQT):
                        for hh in range(2):
                            tp = pool.tile([DH_, P], bf16, name="tp",
                                           tag="job" if pool is jobs else "u")
                            nc.tensor.matmul(
                                tp, lhsT=osbs[qn][:, hh * DH_:(hh + 1) * DH_],
                                rhs=ident, is_transpose=True)
                            dst = ot[hh * DH_:(hh + 1) * DH_,
                                     qn * P:(qn + 1) * P]
                            if act_evict and (qn + hh) % 2 == 0:
                                nc.scalar.copy(dst, tp)
                            else:
                                nc.vector.tensor_copy(dst, tp)

                def yjob(otsb_prev, lqb_prev, dsub, py=None, it0=0,
                         act_evict=False):
                    if VAR >= 3:
                        return
                    if py is None:
                        py = jobs.tile([P, LQB], f32, name="py", tag="job")
                    for it in range(it0, NIT):
                        nc.tensor.matmul(
                            py[:, 0:LQB],
                            lhsT=wot[:, it * D_ + dsub * P:
                                     it * D_ + (dsub + 1) * P],
                            rhs=otsb_prev[it],
                            start=(it == 0), stop=(it == NIT - 1))
                    yo = youts.tile([P, LQB], f32, name="yo")
                    if act_evict:
                        nc.scalar.copy(yo, py[:, 0:LQB])
                    else:
                        nc.vector.tensor_copy(yo, py[:, 0:LQB])
                    nc.sync.dma_start(
                        out=yt[dsub * P:(dsub + 1) * P,
                               lqb_prev * LQB:(lqb_prev + 1) * LQB],
                        in_=yo)

                def yjob_partial(otsb_prev, dsub, pool_tile):
                    if VAR >= 3:
                        return
                    for it in range(NIT - 1):
                        nc.tensor.matmul(
                            pool_tile[:, 0:LQB],
                            lhsT=wot[:, it * D_ + dsub * P:
                                     it * D_ + (dsub + 1) * P],
                            rhs=otsb_prev[it],
                            start=(it == 0), stop=False)

                # block-0 job inserts at (pair, j) slots
                b0_ins = {
                    (0, 0): [("v", 0, 0)], (0, 1): [("v", 0, 1)],
                    (0, 2): [("v", 0, 2)], (0, 3): [("v", 0, 3), ("k", 0, 1)],
                    (0, 4): [("v", 1, 0)], (0, 5): [("v", 1, 1)],
                    (0, 6): [("v", 1, 2), ("k", 0, 2)], (0, 7): [("v", 1, 3)],
                    (0, 8): [("v", 2, 0)], (0, 9): [("v", 2, 1)],
                    (0, 10): [("v", 2, 2), ("k", 0, 3)], (0, 11): [("v", 2, 3)],
                    (0, 12): [("v", 3, 0)], (0, 13): [("v", 3, 1)],
                    (0, 14): [("v", 3, 2), ("q", 0, 1)],
                    (0, 15): [("v", 3, 3), ("k", 1, 0)],
                    (1, 0): [("k", 1, 1)], (1, 2): [("k", 1, 2)],
                    (1, 4): [("k", 1, 3)], (1, 6): [("k", 2, 0)],
                    (1, 8): [("k", 2, 1)], (1, 10): [("k", 2, 2)],
                    (1, 12): [("k", 2, 3)], (1, 14): [("q", 0, 2)],
                    (2, 0): [("k", 3, 0)], (2, 2): [("k", 3, 1)],
                    (2, 4): [("k", 3, 2)], (2, 6): [("k", 3, 3)],
                    (2, 8): [("q", 0, 3)], (2, 12): [("q", 1, 0)],
                    (3, 2): [("q", 1, 1)], (3, 6): [("q", 1, 2)],
                    (3, 10): [("q", 1, 3)],
                }


                # prologue
                kjob(0, 0)
                qjob(0, 0)

                jg = 0
                tdefer = []    # deferred transpose batches
                pend = []      # (hp, et, j, u0, u1, lqb, otsb_cur)
                drains = []    # pairs awaiting drain after last U pop
                ydefer = []
                otsb_blocks = []  # per lqb: dict pair -> ot tile
                partials = []

                for lqb in range(NLQB):
                    if lqb < NLQB - 1:
                        load_x1_slice(lqb + 1)
                    otsb_cur = {hp: otsbp.tile([P, LQB], bf16, name="ot")
                                for hp in range(NPAIR)}
                    otsb_blocks.append(otsb_cur)
                    for hp in range(NPAIR):
                        u0 = upool.tile([P, NQT * VW], f32, name="u0",
                                        tag="u")
                        u1 = upool.tile([P, NQT * VW], f32, name="u1",
                                        tag="u")
                        for j in range(NJT):
                            et = emit_s(hp, j, lqb, jg)
                            jg += 1
                            pend.append((hp, et, j, u0, u1, lqb, otsb_cur))
                            while len(pend) > 4:
                                e = pend.pop(0)
                                emit_u(e[0], e[1], e[2], e[3], e[4])
                                if e[2] == NJT - 1:
                                    drain_pair(e[0], e[3], e[4], e[5], e[6])
                            if j == 9:
                                flush_transposes()
                            if (lqb == NLQB - 1 and hp == NPAIR - 1
                                    and j in (12, 14) and VAR == 0):
                                ds = 4 + (j - 12) // 2
                                pj = jobs.tile([P, LQB], f32, name="py",
                                               tag="job")
                                yjob_partial(
                                    [otsb_blocks[-1][i] for i in range(3)]
                                    + [None], ds, pj)
                                partials.append((ds, pj))
                            if VAR >= 5:
                                pass
                            elif lqb == 0:
                                for spec in b0_ins.get((hp, j), ()):
                                    run_job(spec)
                            else:
                                if j in (10, 13) and ydefer:
                                    yjob(*ydefer.pop(0))
                                if j == 8 and lqb < NLQB - 1:
                                    qjob(lqb + 1, hp)
                        if lqb == NLQB - 1 and hp == NPAIR - 1:
                            # flush: last pair's U + drain
                            while pend:
                                e = pend.pop(0)
                                emit_u(e[0], e[1], e[2], e[3], e[4])
                                if e[2] == NJT - 1:
                                    if e[0] == NPAIR - 1:
                                        # partials on freed st slots
                                        prev = otsb_blocks[-1]
                                        pprev = [prev[i] for i in range(3)]
                                        for sl in range(2):
                                            big = stjp.tile(
                                                [P, 2 * LQB], f32,
                                                name="stt", tag="st")
                                            for hh2 in range(2):
                                                ds = sl * 2 + hh2
                                                pt = big[:, hh2 * LQB:
                                                         (hh2 + 1) * LQB]
                                                yjob_partial(pprev + [None],
                                                             ds, pt)
                                                partials.append((ds, pt))
                                    drain_pair(e[0], e[3], e[4], e[5], e[6])
                    skip = ()
                    if lqb == NLQB - 1:
                        skip = tuple(range(D_ // P))
                    ydefer.extend(
                        ([otsb_blocks[lqb][i] for i in range(NPAIR)], lqb, ds)
                        for ds in range(D_ // P) if ds not in skip)

                while tdefer:
                    flush_transposes(upool, act_evict=True)
                # tail: finish partials (it=3) then remaining yjobs,
                # evictions alternating ACT (idle after last exp) / DVE
                last = [otsb_blocks[-1][i] for i in range(NPAIR)]
                for n, (ds, pt) in enumerate(partials):
                    yjob(last, NLQB - 1, ds, py=pt, it0=NIT - 1,
                         act_evict=(n % 2 == 0))
                for n, ds in enumerate(range(6, D_ // P)):
                    yjob(last, NLQB - 1, ds, act_evict=(n % 2 == 0))
                while ydefer:
                    yjob(*ydefer.pop(0))
    nc.compile()
    return nc


def _get_nc():
    if "nc" not in _CACHE:
        _CACHE["nc"] = _build_nc()
    return _CACHE["nc"]


def _f8():
    import ml_dtypes
    return ml_dtypes.float8_e4m3


def _hi_lo(a):
    f8 = _f8()
    h = a.astype(f8)
    l = (a - h.astype(np.float32)).astype(f8)
    return h, l


def _pack_x(xt):
    # xt [D, L] f32 (already scaled): -> [128, NXS*2*NDT*XS] fp8 hi|lo
    NXS, XS, NDT, P = 4, 512, 8, 128
    v = xt.reshape(NDT, P, NXS, XS).transpose(1, 2, 0, 3)  # p s d c
    h, l = _hi_lo(v)
    out = np.stack([h, l], axis=2)  # p s z d c
    return np.ascontiguousarray(out.reshape(P, NXS * 2 * NDT * XS))


def _pack_w_mmajor(w):
    # w [K, N] f32 scaled -> [128, NIT*2*NDT*128] fp8, m-major hi|lo
    K, N = w.shape
    v = (w.reshape(K // 128, 128, N // 128, 128)   # d p m c
         .transpose(1, 2, 0, 3))                   # p m d c
    h, l = _hi_lo(v)
    out = np.stack([h, l], axis=2)                 # p m z d c
    return np.ascontiguousarray(out.reshape(128, (N // 128) * 2 * (K // 128) * 128))


def _pack_wv(w):
    # w [K, N] f32 scaled -> [128, 2*NDT*N] fp8 hi|lo
    K, N = w.shape
    v = w.reshape(K // 128, 128, N).transpose(1, 0, 2)  # p d c
    h, l = _hi_lo(v)
    out = np.stack([h, l], axis=1)                      # p z d c
    return np.ascontiguousarray(out.reshape(128, 2 * (K // 128) * N))


def _pack_wo(w):
    import ml_dtypes
    K, N = w.shape
    v = w.reshape(K // 128, 128, N).transpose(1, 0, 2).reshape(128, (K // 128) * N)
    return np.ascontiguousarray(v).astype(ml_dtypes.bfloat16)


def _make_in_maps(x1, x2, Wq, Wkv, Wo):
    x1h = [_pack_x(x1[b].T * XSC) for b in range(B)]
    x2h = [_pack_x(x2[b].T * XSC) for b in range(B)]
    in_maps = []
    for c in range(N_CORES):
        b, t = c // 2, c % 2
        in_maps.append({
            "x1h": x1h[b],
            "x2h": x2h[b],
            "wqh": _pack_w_mmajor(Wq[:, t * IH:(t + 1) * IH] * WSC),
            "wkh": _pack_w_mmajor(Wkv[:, t * IH:(t + 1) * IH] * WSC),
            "wvh": _pack_wv(Wkv[:, INNER + t * IH: INNER + (t + 1) * IH] * WSC),
            "woh": _pack_wo(Wo[t * IH:(t + 1) * IH, :]),
        })
    return in_maps


def kernel(x1, x2, Wq, Wkv, Wo, bo):
    import sys
    if "/opt/trn_rl_repo" not in sys.path:
        sys.path.insert(0, "/opt/trn_rl_repo")
    from concourse.bass_utils import run_bass_kernel_spmd

    x1 = np.asarray(x1, dtype=np.float32)
    x2 = np.asarray(x2, dtype=np.float32)
    Wq = np.asarray(Wq, dtype=np.float32)
    Wkv = np.asarray(Wkv, dtype=np.float32)
    Wo = np.asarray(Wo, dtype=np.float32)
    bo = np.asarray(bo, dtype=np.float32)

    nc = _get_nc()
    res = run_bass_kernel_spmd(nc, _make_in_maps(x1, x2, Wq, Wkv, Wo),
                               list(range(N_CORES)))
    y = np.empty((B, L, D), dtype=np.float32)
    for b in range(B):
        y[b] = (res.results[2 * b]["yt"] + res.results[2 * b + 1]["yt"]).T + bo
    return y
